# revision 21
# baseline (speedup 1.0000x reference)
"""ColBERT late-interaction kernel for 8 Trainium2 NeuronCores — v2.

Math (per reference):
  x = h @ W + b                      (projection, H=768 -> D=128)
  v = x / ||x||_2(seq axis)          (normalize over the SEQUENCE axis)
  sim[q,p,n,l] = <q_v[q,n], p_v[p,l]>
  scores[q,p] = sum_n max_{l valid} sim[q,p,n,l]
  out = concat(pos_scores, neg_scores, axis=1)   # [96, 192]

Sharding: passage-parallel. Every core projects ALL queries and a 1/8 shard
of pos+neg passages (24 batches), computes the full-query x local-passage
score block [96, 24]; the host stitches columns.

v2 design vs v1 (223us):
  - All hidden/weight tensors shipped bf16 (halves HBM traffic to ~14MB).
  - Sum-of-squares: DVE squares the (biased, bf16) projected values at
    the 2x TT rate and one segmented reduce_sum per chunk yields ssq per
    batch; rsqrt = exact DVE reciprocal + ACT Sqrt. This replaces v1's
    120 tiny ACT Square+accum instructions (~41us).
  - PSUM drain (the real bottleneck: ~12.4M fp32 sim elements must pass
    through DVE@0.96GHz or ACT@1.2GHz, 1 elem/lane/cycle) is split:
      V-pairs: DVE 4D reduce_max straight from PSUM.
      P-pairs: ACT copies PSUM->SBUF bf16; DVE TT-max folds twice at
               the 2x bf16 rate and one batched reduce per <=4 pairs.
  - Sim tiles tightly packed: 3 segments of VL[t] (tile live length) per
    PSUM bank, 2 banks per 6-passage tile; the matmul streams only
    3*VL<=498 columns per bank instead of v1's padded layout.
  - sum-over-n accumulated across all 27 row-groups into one persistent
    PSUM tile (start/stop flags) -> single copy out.
  - Passage normalize+mask as two Pool tensor_tensor passes:
    xpn = (xp * mask) * rp (rp broadcast along the token axis).
"""

import numpy as np

B, NQ, LP, H, D = 96, 35, 180, 768, 128
NCORES = 8
PB = B // NCORES          # 12 passage batches per core per side
LOCAL_P = 2 * PB          # 24 local passage batches (pos then neg)
QCOLS = B * NQ            # 3360 query columns
PCOLS = LOCAL_P * LP      # 4320 passage columns
KCH = H // 128            # 6 contraction chunks
NGROUPS = (QCOLS + 127) // 128       # 27 interaction row-groups
BPT = 6                   # passage batches per sim tile
NSIMTILES = LOCAL_P // BPT           # 4

# q projection chunks: (batch_start, n_batches); <=512 cols per bank
QCHUNKS = [(0, 28), (28, 28), (56, 28), (84, 12)]
# p projection chunks: 2 batches = 360 cols, one bank each
NPCH = LOCAL_P // 2       # 12

# drain route per tile: P-pairs (ACT drains PSUM, DVE folds bf16 at 2x)
# dominate; V-pairs (direct DVE reduce from PSUM) fill DVE gaps.
# 22 P / 5 V per 27-group sweep balances ACT vs DVE busy time.
def _route_v(g):
    return g in (4, 12, 20) or g >= 25


def _p_runs():
    """Maximal runs of consecutive P-route groups, as (g0, len) tuples."""
    runs = []
    g = 0
    while g < NGROUPS:
        if not _route_v(g):
            g0 = g
            while g < NGROUPS and not _route_v(g):
                g += 1
            runs.append((g0, g - g0))
        else:
            g += 1
    return runs


def _build(tile_lens):
    import concourse.bacc as bacc
    from concourse import mybir
    from concourse.tile import TileContext

    f32 = mybir.dt.float32
    bf16 = mybir.dt.bfloat16
    Ident = mybir.ActivationFunctionType.Identity
    Sqrt = mybir.ActivationFunctionType.Sqrt
    Square = mybir.ActivationFunctionType.Square
    MUL = mybir.AluOpType.mult
    ADD = mybir.AluOpType.add
    MAXOP = mybir.AluOpType.max
    AX = mybir.AxisListType.X

    nc = bacc.Bacc(target_bir_lowering=False)

    # chunk-major hidden layouts: per chunk, per partition, the KCH k-slices
    # are contiguous (one big DMA descriptor per partition)
    QH = nc.dram_tensor("qh", [128, KCH * QCOLS], bf16, kind="ExternalInput")
    PH = nc.dram_tensor("ph", [128, KCH * PCOLS], bf16, kind="ExternalInput")
    WT = nc.dram_tensor("w", [128, KCH * D], bf16, kind="ExternalInput")
    BT = nc.dram_tensor("bias", [D, 1], f32, kind="ExternalInput")
    ONES = nc.dram_tensor("ones", [128, NGROUPS * B], bf16,
                          kind="ExternalInput")
    MASK = nc.dram_tensor("mask", [128, PCOLS], bf16, kind="ExternalInput")
    OUT = nc.dram_tensor("scores", [B, LOCAL_P], f32, kind="ExternalOutput")

    with TileContext(nc) as tc:
        with (
            tc.tile_pool(name="consts", bufs=1) as consts,
            tc.tile_pool(name="hidq", bufs=3) as hidq,
            tc.tile_pool(name="hidp", bufs=6) as hidp,
            tc.tile_pool(name="xbuf", bufs=1) as xbuf,
            tc.tile_pool(name="stats", bufs=1) as stats,
            tc.tile_pool(name="epi", bufs=2) as epi,
            tc.tile_pool(name="fold0", bufs=3) as fold0p,
            tc.tile_pool(name="fold1", bufs=3) as fold1p,
            tc.tile_pool(name="fold2", bufs=3) as fold2p,
            tc.tile_pool(name="mxp", bufs=1) as mxp,
            tc.tile_pool(name="ps_proj", bufs=2, space="PSUM") as ps_proj,
            tc.tile_pool(name="ps_sim", bufs=3, space="PSUM") as ps_sim,
        ):
            w_t = consts.tile([128, KCH, D], bf16, tag="w")
            nc.sync.dma_start(
                out=w_t[:], in_=WT[:].rearrange("p (k d) -> p k d", d=D)
            )
            b_t = consts.tile([D, 1], f32, tag="b")
            nc.sync.dma_start(out=b_t[:], in_=BT[:])
            mask_t = consts.tile([128, PCOLS], bf16, tag="mask")

            xq = xbuf.tile([128, QCOLS], bf16, tag="xq")
            xqn = xbuf.tile([128, QCOLS], bf16, tag="xqn")
            xp = xbuf.tile([128, PCOLS], bf16, tag="xp")
            xpn = xbuf.tile([128, PCOLS], bf16, tag="xpn")
            ssq_q = stats.tile([128, B], f32, tag="ssq_q")
            ssq_p = stats.tile([128, LOCAL_P], f32, tag="ssq_p")
            rq = stats.tile([128, B], bf16, tag="rq")
            rp = stats.tile([128, LOCAL_P], bf16, tag="rp")
            # per-group MaxSim results, all groups side by side
            mx = mxp.tile([128, NGROUPS, LOCAL_P], bf16, tag="mx")

            def q_chunk(c, sq=None):
                qb0, nb = QCHUNKS[c]
                ncols = nb * NQ
                nbank = (ncols + 511) // 512
                bw = ncols // nbank          # 490 or 420, multiple of NQ
                hid = hidq.tile([128, KCH, 28 * NQ], bf16, tag="hid")
                hv = hid[:, :, :ncols]
                nc.sync.dma_start(
                    out=hv,
                    in_=QH[:, KCH * NQ * qb0:KCH * NQ * (qb0 + nb)]
                    .rearrange("p (k n) -> p k n", k=KCH),
                )
                for j in range(nbank):
                    ps = ps_proj.tile([128, 512], f32, tag="proj")
                    for k in range(KCH):
                        nc.tensor.matmul(
                            ps[:, :bw], w_t[:, k, :],
                            hv[:, k, j * bw:(j + 1) * bw],
                            start=(k == 0), stop=(k == KCH - 1),
                        )
                    nc.scalar.activation(
                        xq[:, NQ * qb0 + j * bw:NQ * qb0 + (j + 1) * bw],
                        ps[:, :bw], Ident, bias=b_t[:, 0:1],
                    )
                # ssq from the (biased, bf16) projected values on DVE:
                # square at the 2x bf16 TT rate, one segmented reduce
                xsq = epi.tile([128, 28 * NQ], bf16, tag="xsq")
                lo = NQ * qb0
                (sq or nc.gpsimd).tensor_tensor(
                    out=xsq[:, :ncols], in0=xq[:, lo:lo + ncols],
                    in1=xq[:, lo:lo + ncols], op=MUL,
                )
                nc.vector.reduce_sum(
                    ssq_q[:, qb0:qb0 + nb],
                    xsq[:, :ncols].rearrange("p (n l) -> p n l", l=NQ),
                    axis=AX,
                )

            def p_chunk(c, sq=None):
                pb0 = 2 * c
                lo = LP * pb0
                hid = hidp.tile([128, KCH, 2 * LP], bf16, tag="hid")
                nc.sync.dma_start(
                    out=hid[:],
                    in_=PH[:, KCH * LP * pb0:KCH * LP * (pb0 + 2)]
                    .rearrange("p (k n) -> p k n", k=KCH),
                )
                ps = ps_proj.tile([128, 512], f32, tag="proj")
                for k in range(KCH):
                    nc.tensor.matmul(
                        ps[:, :2 * LP], w_t[:, k, :], hid[:, k, :],
                        start=(k == 0), stop=(k == KCH - 1),
                    )
                nc.scalar.activation(
                    xp[:, lo:lo + 2 * LP],
                    ps[:, :2 * LP], Ident, bias=b_t[:, 0:1],
                )
                # mask-multiply now: it doesn't depend on the norm scale,
                # so it runs in parallel with the ssq -> rsqrt chain
                nc.gpsimd.tensor_tensor(
                    out=xpn[:, lo:lo + 2 * LP], in0=xp[:, lo:lo + 2 * LP],
                    in1=mask_t[:, lo:lo + 2 * LP], op=MUL,
                )
                xsq = epi.tile([128, 28 * NQ], bf16, tag="xsq")
                (sq or nc.gpsimd).tensor_tensor(
                    out=xsq[:, :2 * LP], in0=xp[:, lo:lo + 2 * LP],
                    in1=xp[:, lo:lo + 2 * LP], op=MUL,
                )
                nc.vector.reduce_sum(
                    ssq_p[:, pb0:pb0 + 2],
                    xsq[:, :2 * LP].rearrange("p (n l) -> p n l", l=LP),
                    axis=AX,
                )

            def epilogue(ssq, r, b0, nb):
                """r[:, b0:b0+nb] = rsqrt(ssq): exact DVE reciprocal then
                ACT Sqrt (the fused Rsqrt ACT table is blocked for accuracy)."""
                inv = epi.tile([128, nb], f32, tag="epi_i", name=f"i{b0}_{nb}")
                nc.vector.reciprocal(inv[:], ssq[:, b0:b0 + nb])
                nc.scalar.activation(r[:, b0:b0 + nb], inv[:], Sqrt)

            def q_normalize(c):
                qb0, nb = QCHUNKS[c]
                nc.gpsimd.tensor_tensor(
                    out=xqn[:, NQ * qb0:NQ * (qb0 + nb)].rearrange(
                        "p (n l) -> p n l", l=NQ),
                    in0=xq[:, NQ * qb0:NQ * (qb0 + nb)].rearrange(
                        "p (n l) -> p n l", l=NQ),
                    in1=rq[:, qb0:qb0 + nb].to_broadcast([128, nb, NQ]),
                    op=MUL,
                )

            def p_normalize(c):
                """Scale the (already masked) xpn chunk by rp on Pool."""
                pb0 = 2 * c
                lo = LP * pb0
                nc.gpsimd.tensor_tensor(
                    out=xpn[:, lo:lo + 2 * LP].rearrange(
                        "p (n l) -> p n l", l=LP),
                    in0=xpn[:, lo:lo + 2 * LP].rearrange(
                        "p (n l) -> p n l", l=LP),
                    in1=rp[:, pb0:pb0 + 2].to_broadcast([128, 2, LP]),
                    op=MUL,
                )

            # ---- interaction machinery ----
            def sim_pair(g, t):
                """Matmul the [rows x 6-passage-tile] sim block; 2 banks of
                3 tightly packed VL-long segments each."""
                rows = min(128, QCOLS - g * 128)
                vl = tile_lens[t]
                lhs = xqn[:, g * 128:g * 128 + rows]
                sim = ps_sim.tile([128, 2, 512], f32, tag="sim")
                for j in range(2):
                    p0 = (t * BPT + 3 * j) * LP
                    nc.tensor.matmul(
                        sim[:rows, j, :3 * vl], lhs,
                        xpn[:, p0:p0 + 3 * LP].rearrange(
                            "p (s l) -> p s l", l=LP)[:, :, :vl],
                        start=True, stop=True,
                    )
                return sim, rows, vl

            def drain_v(sim, rows, vl, g, t):
                nc.vector.reduce_max(
                    mx[:rows, g, t * BPT:(t + 1) * BPT],
                    sim[:rows, :, :3 * vl].rearrange(
                        "p j (s l) -> p j s l", l=vl),
                    axis=AX,
                )

            def drain_p_copy(sim, rows, vl, slot, f0):
                """ACT drains one pair's PSUM to bf16 SBUF (ring slot)."""
                nc.scalar.activation(
                    f0[:rows, slot].rearrange(
                        "p (j s) l -> p j s l", j=2)[:, :, :, :vl],
                    sim[:rows, :, :3 * vl].rearrange(
                        "p j (s l) -> p j s l", l=vl),
                    Ident,
                )

            def drain_p_finish(f0, f1, f2, cnt, vl, g, t):
                """DVE: two bf16 TT-max folds at the 2x rate over <=4 pairs,
                then one batched reduce into mx. All widths even so every
                operand stays 4B-aligned for 2x_1P."""
                h1 = ((vl + 3) // 4) * 2
                h2 = ((h1 + 3) // 4) * 2
                nc.vector.tensor_tensor(
                    out=f1[:, :cnt, :, :h1], in0=f0[:, :cnt, :, :h1],
                    in1=f0[:, :cnt, :, vl - h1:vl], op=MAXOP,
                )
                nc.vector.tensor_tensor(
                    out=f2[:, :cnt, :, :h2], in0=f1[:, :cnt, :, :h2],
                    in1=f1[:, :cnt, :, h1 - h2:h1], op=MAXOP,
                )
                nc.vector.reduce_max(
                    mx[:, g - cnt:g, t * BPT:(t + 1) * BPT],
                    f2[:, :cnt, :, :h2],
                    axis=AX,
                )

            def sweep(t, inject=(), post_g=None):
                """Emit all 27 (g, t) pairs; P-runs are drained by ACT into
                bf16 rings and finished by DVE in batches of <=4 pairs.
                `inject` maps g -> emitters to run right before that g;
                `post_g(g)` runs after group g's drain is emitted."""
                vl = tile_lens[t]
                runs = {g0: ln for g0, ln in _p_runs()}
                g = 0
                inj = dict(inject)
                while g < NGROUPS:
                    for fn in inj.pop(g, ()):
                        fn()
                    if _route_v(g):
                        sim, rows, _ = sim_pair(g, t)
                        drain_v(sim, rows, vl, g, t)
                        g += 1
                        if post_g:
                            post_g(g - 1)
                        continue
                    # start of a P run
                    ln = runs[g]
                    nbat = (ln + 3) // 4
                    g0 = g
                    for bi in range(nbat):
                        cnt = min(4, ln - 4 * bi)
                        f0 = fold0p.tile([128, 4, 6 * 168], bf16, tag="f0")
                        f0v = f0[:].rearrange("p s (x l) -> p s x l", l=168)
                        f1 = fold1p.tile([128, 4, 6, 84], bf16, tag="f1")
                        f2 = fold2p.tile([128, 4, 6, 44], bf16, tag="f2")
                        for s in range(cnt):
                            for fn in inj.pop(g, ()):
                                fn()
                            sim, rows, _ = sim_pair(g, t)
                            drain_p_copy(sim, rows, vl, s, f0v)
                            g += 1
                        drain_p_finish(f0v, f1, f2, cnt, vl, g, t)
                        if post_g:
                            for gg in range(g - cnt, g):
                                post_g(gg)
                    assert g == g0 + ln

            def q_full(c, sq=None):
                qb0, nb = QCHUNKS[c]
                q_chunk(c, sq)
                epilogue(ssq_q, rq, qb0, nb)
                q_normalize(c)

            # ---------------- schedule ----------------
            # prologue: q chunk 0 + p tile-0 chunks; q chunks 1-3 are
            # wavefronted into sweep 0 (group g only needs its own chunk).
            # The mask DMA is emitted after the critical hidden chunks so
            # the first-needed bytes win the bandwidth-limited DMA stream.
            q_full(0, sq=nc.vector)
            nc.sync.dma_start(out=mask_t[:], in_=MASK[:])
            for c in range(3):
                p_chunk(c, sq=nc.vector)
            epilogue(ssq_p, rp, 0, BPT)
            for c in range(3):
                p_normalize(c)

            ones_t = consts.tile([128, NGROUPS, B], bf16, tag="ones")
            nc.sync.dma_start(
                out=ones_t[:],
                in_=ONES[:].rearrange("p (g q) -> p g q", q=B),
            )

            def make_tile_prep(t):
                def prep():
                    epilogue(ssq_p, rp, t * BPT, BPT)
                    for c in range(3 * t, 3 * t + 3):
                        p_normalize(c)
                return prep

            # sum over n: one accumulating matmul per group into a single
            # persistent PSUM tile, fired inside sweep 3 as soon as each
            # group's last tile is drained (ps_proj pool is idle by then)
            nsum_tile = [None]

            def fire_nsum(g):
                if nsum_tile[0] is None:
                    nsum_tile[0] = ps_proj.tile(
                        [B, LOCAL_P], f32, tag="proj", name="nsum")
                rows = min(128, QCOLS - g * 128)
                nc.tensor.matmul(
                    nsum_tile[0][:], ones_t[:rows, g, :], mx[:rows, g, :],
                    start=(g == 0), stop=(g == NGROUPS - 1),
                )

            # sweeps; inject remaining q chunks (sweep 0) and the next
            # tile's projection chunks spread through the current sweep
            for t in range(NSIMTILES):
                inject = {}
                if t == 0:
                    inject[1] = [lambda: q_full(1)]
                    inject[6] = [lambda: q_full(2)]
                    inject[11] = [lambda: q_full(3)]
                if t < NSIMTILES - 1:
                    cs = [3 * (t + 1), 3 * (t + 1) + 1, 3 * (t + 1) + 2]
                    inject.setdefault(14 if t == 0 else 5, []).append(
                        lambda c=cs[0]: p_chunk(c))
                    inject.setdefault(17 if t == 0 else 11, []).append(
                        lambda c=cs[1]: p_chunk(c))
                    inject.setdefault(20 if t == 0 else 17, []).append(
                        lambda c=cs[2]: p_chunk(c))
                    inject[23 if t == 0 else 22] = [make_tile_prep(t + 1)]
                sweep(t, inject,
                      post_g=fire_nsum if t == NSIMTILES - 1 else None)

            res = stats.tile([B, LOCAL_P], f32, tag="res")
            nc.scalar.copy(res[:], nsum_tile[0][:])
            nc.sync.dma_start(out=OUT[:], in_=res[:])

    nc.compile()
    return nc


def _chunk_major(hT):
    """[H, cols] -> [128, KCH*cols] bf16: per partition the KCH k-slices of
    this chunk are contiguous (one big DMA descriptor per partition)."""
    import ml_dtypes
    cols = hT.shape[1]
    v = hT.reshape(KCH, 128, cols)
    out = np.ascontiguousarray(
        v.transpose(1, 0, 2).reshape(128, KCH * cols))
    return out.astype(ml_dtypes.bfloat16)


def _prepare(q_hidden, pos_hidden, neg_hidden, W, b, pos_mask, neg_mask):
    import ml_dtypes

    # q: k-major per QCHUNKS chunk so each chunk DMA is 128 contiguous runs
    qhT = np.asarray(q_hidden, np.float32).transpose(2, 0, 1).reshape(
        H, QCOLS)                                          # [H, B*NQ]
    qh_c = np.empty((128, KCH * QCOLS), dtype=ml_dtypes.bfloat16)
    for qb0, nb in QCHUNKS:
        qh_c[:, KCH * NQ * qb0:KCH * NQ * (qb0 + nb)] = _chunk_major(
            np.ascontiguousarray(qhT[:, NQ * qb0:NQ * (qb0 + nb)]))

    Wc = np.ascontiguousarray(
        np.asarray(W, np.float32).reshape(KCH, 128, D)
        .transpose(1, 0, 2).reshape(128, KCH * D)
    ).astype(ml_dtypes.bfloat16)
    bc = np.ascontiguousarray(b, np.float32).reshape(D, 1)

    ones = np.zeros((128, NGROUPS * B), dtype=np.float32)
    for g in range(NGROUPS):
        rows = min(128, QCOLS - g * 128)
        for r in range(rows):
            ones[r, g * B + (g * 128 + r) // NQ] = 1.0
    ones = ones.astype(ml_dtypes.bfloat16)

    per_core = []
    all_V = np.zeros((NCORES, LOCAL_P), dtype=np.int64)
    for i in range(NCORES):
        sl = slice(i * PB, (i + 1) * PB)
        h_loc = np.concatenate([pos_hidden[sl], neg_hidden[sl]], axis=0)
        m_loc = np.concatenate([pos_mask[sl], neg_mask[sl]], axis=0)
        V = m_loc.sum(axis=1).astype(np.int64)
        order = np.argsort(-V, kind="stable")
        phT = np.empty((H, PCOLS), dtype=np.float32)
        mrow = np.empty(PCOLS, dtype=np.float32)
        for j, lb in enumerate(order):
            perm = np.concatenate(
                [np.flatnonzero(m_loc[lb]), np.flatnonzero(~m_loc[lb])]
            )
            phT[:, j * LP:(j + 1) * LP] = h_loc[lb][perm].T
            mrow[j * LP:(j + 1) * LP] = m_loc[lb][perm]
        all_V[i] = V[order]
        ph_c = np.empty((128, KCH * PCOLS), dtype=ml_dtypes.bfloat16)
        for c in range(NPCH):
            pb0 = 2 * c
            ph_c[:, KCH * LP * pb0:KCH * LP * (pb0 + 2)] = _chunk_major(
                np.ascontiguousarray(phT[:, pb0 * LP:(pb0 + 2) * LP]))
        mask_full = np.ascontiguousarray(
            np.broadcast_to(mrow[None, :], (128, PCOLS))
        ).astype(ml_dtypes.bfloat16)
        per_core.append((ph_c, order, mask_full))

    tile_lens = []
    for t in range(NSIMTILES):
        vl = int(all_V[:, t * BPT:(t + 1) * BPT].max())
        vl += vl % 2              # even, for 4B-aligned bf16 2x folds
        assert vl <= 170, f"tile {t} live length {vl} > 170"
        tile_lens.append(vl)

    in_maps = []
    orders = []
    for i in range(NCORES):
        ph_c, order, mask_full = per_core[i]
        in_maps.append({
            "qh": qh_c, "ph": ph_c, "w": Wc, "bias": bc,
            "ones": ones, "mask": mask_full,
        })
        orders.append(order)
    return in_maps, orders, tile_lens


def _assemble(results, orders):
    out = np.zeros((B, 2 * B), dtype=np.float32)
    for i in range(NCORES):
        sc = results[i]["scores"]                          # [96, 24]
        for j, lb in enumerate(orders[i]):
            if lb < PB:
                out[:, i * PB + lb] = sc[:, j]
            else:
                out[:, B + i * PB + (lb - PB)] = sc[:, j]
    return out


def _run(inputs, trace=False):
    from concourse.bass_utils import run_bass_kernel_spmd

    in_maps, orders, tile_lens = _prepare(**inputs)
    nc = _build(tuple(tile_lens))
    res = run_bass_kernel_spmd(nc, in_maps, list(range(NCORES)), trace=trace)
    return _assemble(res.results, orders), res


def kernel(**inputs) -> np.ndarray:
    out, _ = _run(inputs, trace=False)
    return out


def kernel_profiled(**inputs):
    out, res = _run(inputs, trace=True)
    return out, res


# revision 22
# speedup vs baseline: 1.0165x; 1.0165x over previous
"""ColBERT late-interaction kernel for 8 Trainium2 NeuronCores — v2.

Math (per reference):
  x = h @ W + b                      (projection, H=768 -> D=128)
  v = x / ||x||_2(seq axis)          (normalize over the SEQUENCE axis)
  sim[q,p,n,l] = <q_v[q,n], p_v[p,l]>
  scores[q,p] = sum_n max_{l valid} sim[q,p,n,l]
  out = concat(pos_scores, neg_scores, axis=1)   # [96, 192]

Sharding: passage-parallel. Every core projects ALL queries and a 1/8 shard
of pos+neg passages (24 batches), computes the full-query x local-passage
score block [96, 24]; the host stitches columns.

v2 design (223us -> ~148us). Constraints discovered on this toolchain:
only DVE can compute max at all (Pool/GpSimd TensorTensor max fails
codegen; ACT has no max; the fused Rsqrt ACT table is blocked), PSUM is
readable only by DVE (0.96GHz) and ACT (1.2GHz) at 1 elem/lane/cycle,
and tensor_reduce is always 1x while bf16 SBUF TT runs at 2x_1P. Hence:
  - All hidden/weight tensors shipped bf16 (halves HBM traffic to ~14MB).
  - The MaxSim PSUM drain (~12.4M fp32 sim elements/core, the hard
    bottleneck) is split across both PSUM-capable engines:
      V-pairs (5/27): DVE 4D reduce_max straight from PSUM.
      P-pairs (22/27): ACT drains PSUM->SBUF bf16 (Identity copy); DVE
        then TT-max folds twice at the 2x bf16 rate and finishes <=4
        consecutive pairs with ONE batched reduce (amortizes the ~220ns
        DVE fixed cost); widths kept even for 2x_1P's 4B alignment.
  - Sim tiles tightly packed: 3 segments of VL[t] (tile live length,
    rounded even) per PSUM bank, 2 banks per 6-passage tile; the matmul
    streams only 3*VL<=498 columns per bank.
  - Sum-of-squares: squares via TT (x*x) at 2x on DVE for prologue
    chunks / Pool for steady-state ones, one segmented DVE reduce_sum
    per chunk; rsqrt = exact DVE reciprocal + ACT Sqrt (low-precision
    table is fine under the 2e-2 gate). Replaces v1's 120 tiny ACT
    Square+accum instructions (~41us of ACT time).
  - Normalize on Pool (the only engine with spare cycles): q chunks as
    TT with broadcast rq; passages as mask-TT (emitted right after
    projection, off the rsqrt critical path) then broadcast-rp TT.
  - q chunks 1-3 and p chunks 3-11 are wavefronted into the sweeps
    (group g only needs its own chunk), shrinking the DMA-bound
    prologue; sum-over-n matmuls accumulate across all 27 row-groups
    into one persistent PSUM tile and fire inside the last sweep.
"""

import numpy as np

B, NQ, LP, H, D = 96, 35, 180, 768, 128
NCORES = 8
PB = B // NCORES          # 12 passage batches per core per side
LOCAL_P = 2 * PB          # 24 local passage batches (pos then neg)
QCOLS = B * NQ            # 3360 query columns
PCOLS = LOCAL_P * LP      # 4320 passage columns
KCH = H // 128            # 6 contraction chunks
NGROUPS = (QCOLS + 127) // 128       # 27 interaction row-groups
BPT = 6                   # passage batches per sim tile
NSIMTILES = LOCAL_P // BPT           # 4

# q projection chunks: (batch_start, n_batches); <=512 cols per bank
QCHUNKS = [(0, 28), (28, 28), (56, 28), (84, 12)]
# p projection chunks: 2 batches = 360 cols, one bank each
NPCH = LOCAL_P // 2       # 12

# drain route per tile: P-pairs (ACT drains PSUM, DVE folds bf16 at 2x)
# dominate; V-pairs (direct DVE reduce from PSUM) fill DVE gaps.
# 22 P / 5 V per 27-group sweep balances ACT vs DVE busy time.
def _route_v(g):
    return g in (4, 12, 20) or g >= 25


def _p_runs():
    """Maximal runs of consecutive P-route groups, as (g0, len) tuples."""
    runs = []
    g = 0
    while g < NGROUPS:
        if not _route_v(g):
            g0 = g
            while g < NGROUPS and not _route_v(g):
                g += 1
            runs.append((g0, g - g0))
        else:
            g += 1
    return runs


def _build(tile_lens):
    import concourse.bacc as bacc
    from concourse import mybir
    from concourse.tile import TileContext

    f32 = mybir.dt.float32
    bf16 = mybir.dt.bfloat16
    Ident = mybir.ActivationFunctionType.Identity
    Sqrt = mybir.ActivationFunctionType.Sqrt
    Square = mybir.ActivationFunctionType.Square
    MUL = mybir.AluOpType.mult
    ADD = mybir.AluOpType.add
    MAXOP = mybir.AluOpType.max
    AX = mybir.AxisListType.X

    nc = bacc.Bacc(target_bir_lowering=False)

    # chunk-major hidden layouts: per chunk, per partition, the KCH k-slices
    # are contiguous (one big DMA descriptor per partition)
    QH = nc.dram_tensor("qh", [128, KCH * QCOLS], bf16, kind="ExternalInput")
    PH = nc.dram_tensor("ph", [128, KCH * PCOLS], bf16, kind="ExternalInput")
    WT = nc.dram_tensor("w", [128, KCH * D], bf16, kind="ExternalInput")
    BT = nc.dram_tensor("bias", [D, 1], f32, kind="ExternalInput")
    ONES = nc.dram_tensor("ones", [128, NGROUPS * B], bf16,
                          kind="ExternalInput")
    MASK = nc.dram_tensor("mask", [128, PCOLS], bf16, kind="ExternalInput")
    OUT = nc.dram_tensor("scores", [B, LOCAL_P], f32, kind="ExternalOutput")

    with TileContext(nc) as tc:
        with (
            tc.tile_pool(name="consts", bufs=1) as consts,
            tc.tile_pool(name="hidq", bufs=3) as hidq,
            tc.tile_pool(name="hidp", bufs=6) as hidp,
            tc.tile_pool(name="xbuf", bufs=1) as xbuf,
            tc.tile_pool(name="stats", bufs=1) as stats,
            tc.tile_pool(name="epi", bufs=2) as epi,
            tc.tile_pool(name="fold0", bufs=3) as fold0p,
            tc.tile_pool(name="fold1", bufs=3) as fold1p,
            tc.tile_pool(name="fold2", bufs=3) as fold2p,
            tc.tile_pool(name="mxp", bufs=1) as mxp,
            tc.tile_pool(name="ps_proj", bufs=2, space="PSUM") as ps_proj,
            tc.tile_pool(name="ps_sim", bufs=3, space="PSUM") as ps_sim,
        ):
            w_t = consts.tile([128, KCH, D], bf16, tag="w")
            nc.sync.dma_start(
                out=w_t[:], in_=WT[:].rearrange("p (k d) -> p k d", d=D)
            )
            b_t = consts.tile([D, 1], f32, tag="b")
            nc.sync.dma_start(out=b_t[:], in_=BT[:])
            mask_t = consts.tile([128, PCOLS], bf16, tag="mask")

            xq = xbuf.tile([128, QCOLS], bf16, tag="xq")
            xqn = xbuf.tile([128, QCOLS], bf16, tag="xqn")
            xp = xbuf.tile([128, PCOLS], bf16, tag="xp")
            xpn = xbuf.tile([128, PCOLS], bf16, tag="xpn")
            ssq_q = stats.tile([128, B], f32, tag="ssq_q")
            ssq_p = stats.tile([128, LOCAL_P], f32, tag="ssq_p")
            rq = stats.tile([128, B], bf16, tag="rq")
            rp = stats.tile([128, LOCAL_P], bf16, tag="rp")
            # per-group MaxSim results, all groups side by side
            mx = mxp.tile([128, NGROUPS, LOCAL_P], bf16, tag="mx")

            def q_chunk(c, sq=None):
                qb0, nb = QCHUNKS[c]
                ncols = nb * NQ
                nbank = (ncols + 511) // 512
                bw = ncols // nbank          # 490 or 420, multiple of NQ
                hid = hidq.tile([128, KCH, 28 * NQ], bf16, tag="hid")
                hv = hid[:, :, :ncols]
                nc.sync.dma_start(
                    out=hv,
                    in_=QH[:, KCH * NQ * qb0:KCH * NQ * (qb0 + nb)]
                    .rearrange("p (k n) -> p k n", k=KCH),
                )
                for j in range(nbank):
                    ps = ps_proj.tile([128, 512], f32, tag="proj")
                    for k in range(KCH):
                        nc.tensor.matmul(
                            ps[:, :bw], w_t[:, k, :],
                            hv[:, k, j * bw:(j + 1) * bw],
                            start=(k == 0), stop=(k == KCH - 1),
                        )
                    nc.scalar.activation(
                        xq[:, NQ * qb0 + j * bw:NQ * qb0 + (j + 1) * bw],
                        ps[:, :bw], Ident, bias=b_t[:, 0:1],
                    )
                # ssq from the (biased, bf16) projected values on DVE:
                # square at the 2x bf16 TT rate, one segmented reduce
                xsq = epi.tile([128, 28 * NQ], bf16, tag="xsq")
                lo = NQ * qb0
                (sq or nc.gpsimd).tensor_tensor(
                    out=xsq[:, :ncols], in0=xq[:, lo:lo + ncols],
                    in1=xq[:, lo:lo + ncols], op=MUL,
                )
                nc.vector.reduce_sum(
                    ssq_q[:, qb0:qb0 + nb],
                    xsq[:, :ncols].rearrange("p (n l) -> p n l", l=NQ),
                    axis=AX,
                )

            def p_chunk(c, sq=None):
                pb0 = 2 * c
                lo = LP * pb0
                hid = hidp.tile([128, KCH, 2 * LP], bf16, tag="hid")
                nc.sync.dma_start(
                    out=hid[:],
                    in_=PH[:, KCH * LP * pb0:KCH * LP * (pb0 + 2)]
                    .rearrange("p (k n) -> p k n", k=KCH),
                )
                ps = ps_proj.tile([128, 512], f32, tag="proj")
                for k in range(KCH):
                    nc.tensor.matmul(
                        ps[:, :2 * LP], w_t[:, k, :], hid[:, k, :],
                        start=(k == 0), stop=(k == KCH - 1),
                    )
                nc.scalar.activation(
                    xp[:, lo:lo + 2 * LP],
                    ps[:, :2 * LP], Ident, bias=b_t[:, 0:1],
                )
                # mask-multiply now: it doesn't depend on the norm scale,
                # so it runs in parallel with the ssq -> rsqrt chain
                nc.gpsimd.tensor_tensor(
                    out=xpn[:, lo:lo + 2 * LP], in0=xp[:, lo:lo + 2 * LP],
                    in1=mask_t[:, lo:lo + 2 * LP], op=MUL,
                )
                xsq = epi.tile([128, 28 * NQ], bf16, tag="xsq")
                (sq or nc.gpsimd).tensor_tensor(
                    out=xsq[:, :2 * LP], in0=xp[:, lo:lo + 2 * LP],
                    in1=xp[:, lo:lo + 2 * LP], op=MUL,
                )
                nc.vector.reduce_sum(
                    ssq_p[:, pb0:pb0 + 2],
                    xsq[:, :2 * LP].rearrange("p (n l) -> p n l", l=LP),
                    axis=AX,
                )

            def epilogue(ssq, r, b0, nb):
                """r[:, b0:b0+nb] = rsqrt(ssq): exact DVE reciprocal then
                ACT Sqrt (the fused Rsqrt ACT table is blocked for accuracy)."""
                inv = epi.tile([128, nb], f32, tag="epi_i", name=f"i{b0}_{nb}")
                nc.vector.reciprocal(inv[:], ssq[:, b0:b0 + nb])
                nc.scalar.activation(r[:, b0:b0 + nb], inv[:], Sqrt)

            def q_normalize(c):
                qb0, nb = QCHUNKS[c]
                nc.gpsimd.tensor_tensor(
                    out=xqn[:, NQ * qb0:NQ * (qb0 + nb)].rearrange(
                        "p (n l) -> p n l", l=NQ),
                    in0=xq[:, NQ * qb0:NQ * (qb0 + nb)].rearrange(
                        "p (n l) -> p n l", l=NQ),
                    in1=rq[:, qb0:qb0 + nb].to_broadcast([128, nb, NQ]),
                    op=MUL,
                )

            def p_normalize(c):
                """Scale the (already masked) xpn chunk by rp on Pool."""
                pb0 = 2 * c
                lo = LP * pb0
                nc.gpsimd.tensor_tensor(
                    out=xpn[:, lo:lo + 2 * LP].rearrange(
                        "p (n l) -> p n l", l=LP),
                    in0=xpn[:, lo:lo + 2 * LP].rearrange(
                        "p (n l) -> p n l", l=LP),
                    in1=rp[:, pb0:pb0 + 2].to_broadcast([128, 2, LP]),
                    op=MUL,
                )

            # ---- interaction machinery ----
            def sim_pair(g, t):
                """Matmul the [rows x 6-passage-tile] sim block; 2 banks of
                3 tightly packed VL-long segments each."""
                rows = min(128, QCOLS - g * 128)
                vl = tile_lens[t]
                lhs = xqn[:, g * 128:g * 128 + rows]
                sim = ps_sim.tile([128, 2, 512], f32, tag="sim")
                for j in range(2):
                    p0 = (t * BPT + 3 * j) * LP
                    nc.tensor.matmul(
                        sim[:rows, j, :3 * vl], lhs,
                        xpn[:, p0:p0 + 3 * LP].rearrange(
                            "p (s l) -> p s l", l=LP)[:, :, :vl],
                        start=True, stop=True,
                    )
                return sim, rows, vl

            def drain_v(sim, rows, vl, g, t):
                nc.vector.reduce_max(
                    mx[:rows, g, t * BPT:(t + 1) * BPT],
                    sim[:rows, :, :3 * vl].rearrange(
                        "p j (s l) -> p j s l", l=vl),
                    axis=AX,
                )

            def drain_p_copy(sim, rows, vl, slot, f0):
                """ACT drains one pair's PSUM to bf16 SBUF (ring slot)."""
                nc.scalar.activation(
                    f0[:rows, slot].rearrange(
                        "p (j s) l -> p j s l", j=2)[:, :, :, :vl],
                    sim[:rows, :, :3 * vl].rearrange(
                        "p j (s l) -> p j s l", l=vl),
                    Ident,
                )

            def drain_p_finish(f0, f1, f2, cnt, vl, g, t):
                """DVE: two bf16 TT-max folds at the 2x rate over <=4 pairs,
                then one batched reduce into mx. All widths even so every
                operand stays 4B-aligned for 2x_1P."""
                h1 = ((vl + 3) // 4) * 2
                h2 = ((h1 + 3) // 4) * 2
                nc.vector.tensor_tensor(
                    out=f1[:, :cnt, :, :h1], in0=f0[:, :cnt, :, :h1],
                    in1=f0[:, :cnt, :, vl - h1:vl], op=MAXOP,
                )
                nc.vector.tensor_tensor(
                    out=f2[:, :cnt, :, :h2], in0=f1[:, :cnt, :, :h2],
                    in1=f1[:, :cnt, :, h1 - h2:h1], op=MAXOP,
                )
                nc.vector.reduce_max(
                    mx[:, g - cnt:g, t * BPT:(t + 1) * BPT],
                    f2[:, :cnt, :, :h2],
                    axis=AX,
                )

            def sweep(t, inject=(), post_g=None):
                """Emit all 27 (g, t) pairs; P-runs are drained by ACT into
                bf16 rings and finished by DVE in batches of <=4 pairs.
                `inject` maps g -> emitters to run right before that g;
                `post_g(g)` runs after group g's drain is emitted."""
                vl = tile_lens[t]
                runs = {g0: ln for g0, ln in _p_runs()}
                g = 0
                inj = dict(inject)
                while g < NGROUPS:
                    for fn in inj.pop(g, ()):
                        fn()
                    if _route_v(g):
                        sim, rows, _ = sim_pair(g, t)
                        drain_v(sim, rows, vl, g, t)
                        g += 1
                        if post_g:
                            post_g(g - 1)
                        continue
                    # start of a P run
                    ln = runs[g]
                    nbat = (ln + 3) // 4
                    g0 = g
                    for bi in range(nbat):
                        cnt = min(4, ln - 4 * bi)
                        f0 = fold0p.tile([128, 4, 6 * 168], bf16, tag="f0")
                        f0v = f0[:].rearrange("p s (x l) -> p s x l", l=168)
                        f1 = fold1p.tile([128, 4, 6, 84], bf16, tag="f1")
                        f2 = fold2p.tile([128, 4, 6, 44], bf16, tag="f2")
                        for s in range(cnt):
                            for fn in inj.pop(g, ()):
                                fn()
                            sim, rows, _ = sim_pair(g, t)
                            drain_p_copy(sim, rows, vl, s, f0v)
                            g += 1
                        drain_p_finish(f0v, f1, f2, cnt, vl, g, t)
                        if post_g:
                            for gg in range(g - cnt, g):
                                post_g(gg)
                    assert g == g0 + ln

            def q_full(c, sq=None):
                qb0, nb = QCHUNKS[c]
                q_chunk(c, sq)
                epilogue(ssq_q, rq, qb0, nb)
                q_normalize(c)

            # ---------------- schedule ----------------
            # prologue: q chunk 0 + p tile-0 chunks; q chunks 1-3 are
            # wavefronted into sweep 0 (group g only needs its own chunk).
            # The mask DMA is emitted after the critical hidden chunks so
            # the first-needed bytes win the bandwidth-limited DMA stream.
            q_full(0, sq=nc.vector)
            nc.sync.dma_start(out=mask_t[:], in_=MASK[:])
            for c in range(3):
                p_chunk(c, sq=nc.vector)
            epilogue(ssq_p, rp, 0, BPT)
            for c in range(3):
                p_normalize(c)

            ones_t = consts.tile([128, NGROUPS, B], bf16, tag="ones")
            nc.sync.dma_start(
                out=ones_t[:],
                in_=ONES[:].rearrange("p (g q) -> p g q", q=B),
            )

            def make_tile_prep(t):
                def prep():
                    epilogue(ssq_p, rp, t * BPT, BPT)
                    for c in range(3 * t, 3 * t + 3):
                        p_normalize(c)
                return prep

            # sum over n: one accumulating matmul per group into a single
            # persistent PSUM tile, fired inside sweep 3 as soon as each
            # group's last tile is drained (ps_proj pool is idle by then)
            nsum_tile = [None]

            def fire_nsum(g):
                if nsum_tile[0] is None:
                    nsum_tile[0] = ps_proj.tile(
                        [B, LOCAL_P], f32, tag="proj", name="nsum")
                rows = min(128, QCOLS - g * 128)
                nc.tensor.matmul(
                    nsum_tile[0][:], ones_t[:rows, g, :], mx[:rows, g, :],
                    start=(g == 0), stop=(g == NGROUPS - 1),
                )

            # sweeps; inject remaining q chunks (sweep 0) and the next
            # tile's projection chunks spread through the current sweep
            for t in range(NSIMTILES):
                inject = {}
                if t == 0:
                    inject[1] = [lambda: q_full(1)]
                    inject[6] = [lambda: q_full(2)]
                    inject[11] = [lambda: q_full(3)]
                if t < NSIMTILES - 1:
                    cs = [3 * (t + 1), 3 * (t + 1) + 1, 3 * (t + 1) + 2]
                    inject.setdefault(14 if t == 0 else 5, []).append(
                        lambda c=cs[0]: p_chunk(c))
                    inject.setdefault(17 if t == 0 else 11, []).append(
                        lambda c=cs[1]: p_chunk(c))
                    inject.setdefault(20 if t == 0 else 17, []).append(
                        lambda c=cs[2]: p_chunk(c))
                    inject[23 if t == 0 else 22] = [make_tile_prep(t + 1)]
                sweep(t, inject,
                      post_g=fire_nsum if t == NSIMTILES - 1 else None)

            res = stats.tile([B, LOCAL_P], f32, tag="res")
            nc.scalar.copy(res[:], nsum_tile[0][:])
            nc.sync.dma_start(out=OUT[:], in_=res[:])

    nc.compile()
    return nc


def _chunk_major(hT):
    """[H, cols] -> [128, KCH*cols] bf16: per partition the KCH k-slices of
    this chunk are contiguous (one big DMA descriptor per partition)."""
    import ml_dtypes
    cols = hT.shape[1]
    v = hT.reshape(KCH, 128, cols)
    out = np.ascontiguousarray(
        v.transpose(1, 0, 2).reshape(128, KCH * cols))
    return out.astype(ml_dtypes.bfloat16)


def _prepare(q_hidden, pos_hidden, neg_hidden, W, b, pos_mask, neg_mask):
    import ml_dtypes

    # q: k-major per QCHUNKS chunk so each chunk DMA is 128 contiguous runs
    qhT = np.asarray(q_hidden, np.float32).transpose(2, 0, 1).reshape(
        H, QCOLS)                                          # [H, B*NQ]
    qh_c = np.empty((128, KCH * QCOLS), dtype=ml_dtypes.bfloat16)
    for qb0, nb in QCHUNKS:
        qh_c[:, KCH * NQ * qb0:KCH * NQ * (qb0 + nb)] = _chunk_major(
            np.ascontiguousarray(qhT[:, NQ * qb0:NQ * (qb0 + nb)]))

    Wc = np.ascontiguousarray(
        np.asarray(W, np.float32).reshape(KCH, 128, D)
        .transpose(1, 0, 2).reshape(128, KCH * D)
    ).astype(ml_dtypes.bfloat16)
    bc = np.ascontiguousarray(b, np.float32).reshape(D, 1)

    ones = np.zeros((128, NGROUPS * B), dtype=np.float32)
    for g in range(NGROUPS):
        rows = min(128, QCOLS - g * 128)
        for r in range(rows):
            ones[r, g * B + (g * 128 + r) // NQ] = 1.0
    ones = ones.astype(ml_dtypes.bfloat16)

    per_core = []
    all_V = np.zeros((NCORES, LOCAL_P), dtype=np.int64)
    for i in range(NCORES):
        sl = slice(i * PB, (i + 1) * PB)
        h_loc = np.concatenate([pos_hidden[sl], neg_hidden[sl]], axis=0)
        m_loc = np.concatenate([pos_mask[sl], neg_mask[sl]], axis=0)
        V = m_loc.sum(axis=1).astype(np.int64)
        order = np.argsort(-V, kind="stable")
        phT = np.empty((H, PCOLS), dtype=np.float32)
        mrow = np.empty(PCOLS, dtype=np.float32)
        for j, lb in enumerate(order):
            perm = np.concatenate(
                [np.flatnonzero(m_loc[lb]), np.flatnonzero(~m_loc[lb])]
            )
            phT[:, j * LP:(j + 1) * LP] = h_loc[lb][perm].T
            mrow[j * LP:(j + 1) * LP] = m_loc[lb][perm]
        all_V[i] = V[order]
        ph_c = np.empty((128, KCH * PCOLS), dtype=ml_dtypes.bfloat16)
        for c in range(NPCH):
            pb0 = 2 * c
            ph_c[:, KCH * LP * pb0:KCH * LP * (pb0 + 2)] = _chunk_major(
                np.ascontiguousarray(phT[:, pb0 * LP:(pb0 + 2) * LP]))
        mask_full = np.ascontiguousarray(
            np.broadcast_to(mrow[None, :], (128, PCOLS))
        ).astype(ml_dtypes.bfloat16)
        per_core.append((ph_c, order, mask_full))

    tile_lens = []
    for t in range(NSIMTILES):
        vl = int(all_V[:, t * BPT:(t + 1) * BPT].max())
        vl += vl % 2              # even, for 4B-aligned bf16 2x folds
        assert vl <= 170, f"tile {t} live length {vl} > 170"
        tile_lens.append(vl)

    in_maps = []
    orders = []
    for i in range(NCORES):
        ph_c, order, mask_full = per_core[i]
        in_maps.append({
            "qh": qh_c, "ph": ph_c, "w": Wc, "bias": bc,
            "ones": ones, "mask": mask_full,
        })
        orders.append(order)
    return in_maps, orders, tile_lens


def _assemble(results, orders):
    out = np.zeros((B, 2 * B), dtype=np.float32)
    for i in range(NCORES):
        sc = results[i]["scores"]                          # [96, 24]
        for j, lb in enumerate(orders[i]):
            if lb < PB:
                out[:, i * PB + lb] = sc[:, j]
            else:
                out[:, B + i * PB + (lb - PB)] = sc[:, j]
    return out


def _run(inputs, trace=False):
    from concourse.bass_utils import run_bass_kernel_spmd

    in_maps, orders, tile_lens = _prepare(**inputs)
    nc = _build(tuple(tile_lens))
    res = run_bass_kernel_spmd(nc, in_maps, list(range(NCORES)), trace=trace)
    return _assemble(res.results, orders), res


def kernel(**inputs) -> np.ndarray:
    out, _ = _run(inputs, trace=False)
    return out


def kernel_profiled(**inputs):
    out, res = _run(inputs, trace=True)
    return out, res
